# revision 42
# baseline (speedup 1.0000x reference)
"""Trainium2 Bass kernel for nn_LogisticRegression (embedding_lookup).

Reference computation (B=1024, S=200, V=50000, E=300):
    x1 = one-hot presence over vocab (duplicates set once)      [B, V]
    emb_mean = mean(emb_table[x], axis=1)                       [B, E]
    logits = concat([emb_mean, x1]) @ W.T + b                   [B, 1]
    out = sigmoid(logits)

Algebraic restructure (never materializes x1 / feats):
    t[v]     = emb_table[v] . W[0, :E] / S
    w[v]     = W[0, E + v]
    logit[i] = sum_v pres[i,v]*(t[v]+w[v]) + sum_dups (cnt-pres)*t[v] + b
where pres is the 0/1 presence matrix and the dup correction covers the
rare tokens repeated within a row (~0.4 per row).

Device plan (single NEFF, SPMD on 8 cores, vocab-sharded):
  Core c owns vocab rows [c*6250, (c+1)*6250) padded to 6400 = 50*128.
  phase 1: stream the 7.7MB table slice (chunks split across the sync
           and vector DMA queues); per column k a fused DVE
           scalar_tensor_tensor computes t[:, k] = sum_e tbl*We/S with
           accum_out; u = 64*(t + wvoc) cast to fp8e4 (the 64x scale
           keeps u in e4m3 normal range; undone in the final sigmoid).
  phase 2: logits as a DENSE matmul against a host-built fp8 presence
           matrix (exact 0/1 values): DoubleRow fp8 matmuls contract 256
           vocab rows per instruction,
              psum[1, 1024] += u8[:, k:k+2].T @ presT[..][128, 2, 1024]
           accumulating across 25 pairs in 2 PSUM banks.  This replaces
           the per-token SWDGE dma_gather of the v1 kernel (which
           serialized ~8.4ns/index on the GpSimd Q7 path = 216us).
           Both streams rotate over the three DMA-capable issue queues
           (sync/scalar/gpsimd, ~95GB/s each; the aggregate ~280GB/s is
           the per-core HBM fair share with all 8 cores streaming), all
           tbl chunks ahead of pres chunks within each queue.
  dup fix: one 128-slot dma_gather pulls 256B blocks of the t table,
           a DVE one-hot extract forms val[s] = 64*d_s*t[p_s], and a
           [128,1].T @ [128,1024] matmul spreads the corrections onto
           the same PSUM accumulators.
  finish:  ReduceScatter(add) of the [1024] partial logits (4KB); core c
           receives rows [128c, 128c+128), applies sigmoid(x/64 + b),
           and writes its 128 outputs.  Host concatenation is a plain
           reorder of integer-indexed slices.

Empirical ground rules for this stack (established by direct HW tests):
  * indirect_dma_start ~10ns/descriptor serialized; dma_gather ~8.4ns/idx
    on the GpSimd software-DGE path -> avoid bulk gathers entirely;
  * tensor_tensor_reduce / tensor_scalar(accum_out) crash the compiled
    NEFF; scalar_tensor_tensor(accum_out) works and is used here;
  * one HWDGE queue sustains ~150GB/s; spread big streams across the
    sync/vector/scalar/gpsimd queues to approach the ~360GB/s core BW.

Host side only shards tensors and precomputes integer index data (the
presence bitmap, dup slots, gather block ids) from the int token ids,
then concatenates the per-core outputs.
"""

import sys

if "/opt/trn_rl_repo" not in sys.path:
    sys.path.insert(0, "/opt/trn_rl_repo")

# This image's antenv package lacks the optional axon_hooks module, but
# concourse.bass_utils imports it unconditionally on the BASS_TRACE path.
# Provide a compatible stub so tracing degrades gracefully instead of
# crashing; a harness may install a real hook via set_axon_ntff_profile_hook.
try:
    import antenv.axon_hooks  # noqa: F401
except ImportError:
    import types as _types

    import antenv as _antenv

    _hooks_mod = _types.ModuleType("antenv.axon_hooks")
    _hooks_mod._hook = None

    def _set_hook(h, _m=_hooks_mod):
        _m._hook = h

    def _get_hook(_m=_hooks_mod):
        return _m._hook

    _hooks_mod.set_axon_ntff_profile_hook = _set_hook
    _hooks_mod.get_axon_ntff_profile_hook = _get_hook
    sys.modules["antenv.axon_hooks"] = _hooks_mod
    _antenv.axon_hooks = _hooks_mod

import ml_dtypes
import numpy as np

from concourse import bacc, bass, mybir, tile
from concourse.bass_utils import run_bass_kernel_spmd

# Problem shapes (hardcoded per contract).
N_CORES = 8
B = 1024
S = 200
V = 50000
E = 300

VPC = V // N_CORES          # vocab rows per core = 6250
KC = 50                     # u columns; padded vocab rows = 128*KC = 6400
VPAD = KC * 128
CHUNKS = [2, 4, 4, 6, 6, 6, 6, 6, 6, 4]   # k-columns per chunk group (even)
DUPN = 128                  # dup-correction slots per core
GBLK = 64                   # f32 per 256B gather block
USCALE = 64.0               # fp8 scale for u (undone in final sigmoid)

_BUILT = None
LAST_RUN = None  # BassKernelResults of the most recent launch (for harness)


def _build():
    f32 = mybir.dt.float32
    bf16 = mybir.dt.bfloat16
    fp8 = mybir.dt.float8e4
    i16 = mybir.dt.int16
    nc = bacc.Bacc("TRN2", target_bir_lowering=False, debug=False,
                   num_devices=N_CORES)

    # tbl/pres are host-pretransposed to partition-major [128, KC, ...] so
    # each streamed chunk is one contiguous run per partition (128 DMA
    # descriptors instead of 128*tch; multi-row APs cost the issuing
    # engine ~2-5us each)
    tbl = nc.dram_tensor("tbl", [128, KC, E], f32, kind="ExternalInput")
    wemb = nc.dram_tensor("wemb", [1, E], f32, kind="ExternalInput")
    wvoc = nc.dram_tensor("wvoc", [128, KC], f32, kind="ExternalInput")
    pres = nc.dram_tensor("pres", [128, KC, B], fp8, kind="ExternalInput")
    didx = nc.dram_tensor("didx", [128, DUPN // 16], i16, kind="ExternalInput")
    dwv = nc.dram_tensor("dwv", [128, 1, GBLK], bf16, kind="ExternalInput")
    dspread = nc.dram_tensor("dspread", [128, B], bf16, kind="ExternalInput")
    bias = nc.dram_tensor("bias", [1, 1], f32, kind="ExternalInput")
    outp = nc.dram_tensor("outp", [1, B // N_CORES], f32, kind="ExternalOutput")

    with tile.TileContext(nc) as tc:
        with tc.tile_pool(name="dram", bufs=1, space="DRAM") as dram, \
             tc.tile_pool(name="sbuf", bufs=1) as sb1, \
             tc.tile_pool(name="tb", bufs=len(CHUNKS)) as tb, \
             tc.tile_pool(name="pr", bufs=len(CHUNKS)) as pr, \
             tc.tile_pool(name="uf", bufs=len(CHUNKS)) as uf, \
             tc.tile_pool(name="scr", bufs=2) as scr, \
             tc.tile_pool(name="ps", bufs=1, space="PSUM") as ps:
            t_dram = dram.tile([VPAD // GBLK, GBLK], f32)
            # partial logits cross the cores as bf16: the CC stream has a
            # fixed ~88us init in this runtime (measured: a warm-up RS with
            # input ready at 3us still executed at 89us), so the reduce is
            # the tail; halving its payload trims the exec slice
            accd = dram.tile([1, B], bf16)
            rsd = dram.tile([1, B // N_CORES], bf16)

            # --- small input loads (spread off the big-stream queues) ---
            wemb_sb = sb1.tile([128, E], f32)
            nc.scalar.dma_start(wemb_sb[:], wemb.ap().partition_broadcast(128))
            # fold the 1/S of the sequence mean into the embedding weights
            nc.vector.tensor_scalar_mul(wemb_sb[:], wemb_sb[:], 1.0 / S)
            wvoc_sb = sb1.tile([128, KC], f32)
            nc.scalar.dma_start(wvoc_sb[:], wvoc.ap())
            didx_sb = sb1.tile([128, DUPN // 16], i16)
            nc.sync.dma_start(didx_sb[:], didx.ap())
            dwv_sb = sb1.tile([128, 1, GBLK], bf16)
            nc.sync.dma_start(dwv_sb[:], dwv.ap())
            b_sb = sb1.tile([1, 1], f32)
            nc.scalar.dma_start(b_sb[:], bias.ap())
            t_raw = sb1.tile([128, KC], f32)
            psA = ps.tile([2, B // 2], f32)
            psB = ps.tile([2, B // 2], f32)

            # --- issue both big streams upfront, rotated over the three
            # DMA-capable queues (sync / scalar / gpsimd); within each
            # queue all tbl chunks sit ahead of pres chunks, since phase 1
            # gates everything downstream
            rot = [nc.sync, nc.scalar, nc.gpsimd]
            tchunks, pchunks = [], []
            k0 = 0
            for ch, tch in enumerate(CHUNKS):
                chunk = tb.tile([128, tch, E], f32, tag="tblchunk")
                rot[ch % 3].dma_start(chunk[:], tbl.ap()[:, k0:k0 + tch, :])
                tchunks.append(chunk)
                k0 += tch
            k0 = 0
            for ch, tch in enumerate(CHUNKS):
                pchunk = pr.tile([128, tch, B], fp8, tag="preschunk")
                rot[(ch + 1) % 3].dma_start(
                    pchunk[:], pres.ap()[:, k0:k0 + tch, :])
                pchunks.append(pchunk)
                k0 += tch
            # prime the Activation engine's sigmoid table now that its DMA
            # issues are queued (the 2.5us ACT_TABLE_LOAD otherwise blocks
            # them, or lands on the tail)
            warm = scr.tile([1, 1], f32, tag="warm")
            nc.scalar.activation(
                out=warm[:], in_=b_sb[:],
                func=mybir.ActivationFunctionType.Sigmoid)

            k0 = 0
            for ch, tch in enumerate(CHUNKS):
                chunk = tchunks[ch]
                pchunk = pchunks[ch]
                for t in range(tch):
                    k = k0 + t
                    po = scr.tile([128, E], f32, tag="po")
                    nc.vector.scalar_tensor_tensor(
                        out=po[:], in0=chunk[:, t, :], scalar=1.0,
                        in1=wemb_sb[:],
                        op0=mybir.AluOpType.mult, op1=mybir.AluOpType.mult,
                        accum_out=t_raw[:, k:k + 1])
                # u = 64*(t + wvoc) for this group, cast to fp8 for the PE.
                # Per-group u tile in DoubleRow weight layout [p, ktile r,
                # local pair c, m]: value u(2c+r) at m=0, zeros at the dummy
                # m=1 column (the dual-fp8 LDWEIGHTS path requires
                # n_elem[2]==2 and a 16B-aligned k-tile stride; the 16-slot
                # c axis gives r-stride 32B).  A single shared tile would
                # serialize this group's cast behind the previous group's
                # matmuls (write-after-read), stalling the DVE on the pres
                # stream.
                u_g = uf.tile([128, 2, 16, 2], fp8, tag="ug")
                nc.gpsimd.memset(u_g[:], 0.0)
                sl = slice(k0, k0 + tch)
                usl = scr.tile([128, tch], f32, tag="usl")
                nc.vector.tensor_tensor(
                    out=usl[:], in0=t_raw[:, sl], in1=wvoc_sb[:, sl],
                    op=mybir.AluOpType.add)
                for t in range(0, tch, 2):
                    nc.vector.tensor_scalar_mul(
                        u_g[:, :, t // 2, 0:1], usl[:, t:t + 2].unsqueeze(2),
                        USCALE)
                for t in range(0, tch, 2):
                    lhs = u_g[:, :, t // 2, :]
                    nc.tensor.matmul(
                        psA[:], lhs, pchunk[:, t:t + 2, 0:B // 2],
                        start=(k0 + t == 0), stop=False,
                        perf_mode=mybir.MatmulPerfMode.DoubleRow)
                    nc.tensor.matmul(
                        psB[:], lhs, pchunk[:, t:t + 2, B // 2:B],
                        start=(k0 + t == 0), stop=False,
                        perf_mode=mybir.MatmulPerfMode.DoubleRow)
                k0 += tch

            # --- dup correction: gather 256B t-blocks, extract, spread ---
            nc.sync.dma_start(
                t_dram[:].rearrange("q e -> (q e)").rearrange("(p k) -> p k", k=KC),
                t_raw[:])
            # dspread arrives late on the gpsimd queue; it is only needed by
            # the closing matmuls below
            dspread_sb = sb1.tile([128, B], bf16)
            nc.gpsimd.dma_start(dspread_sb[:], dspread.ap())
            g = sb1.tile([128, 1, GBLK], f32)
            nc.gpsimd.dma_gather(
                g[:], t_dram[:], didx_sb[:],
                num_idxs=DUPN, num_idxs_reg=DUPN, elem_size=GBLK)
            dpo = scr.tile([128, 1, GBLK], f32, tag="dpo")
            dval = sb1.tile([128, 1], f32)
            nc.vector.scalar_tensor_tensor(
                out=dpo[:], in0=g[:], scalar=1.0, in1=dwv_sb[:],
                op0=mybir.AluOpType.mult, op1=mybir.AluOpType.mult,
                accum_out=dval[:])
            dval_bf = sb1.tile([128, 1], bf16)
            nc.vector.tensor_copy(out=dval_bf[:], in_=dval[:])
            nc.tensor.matmul(psA[0:1, :], dval_bf[:], dspread_sb[:, 0:B // 2],
                             start=False, stop=True)
            nc.tensor.matmul(psB[0:1, :], dval_bf[:], dspread_sb[:, B // 2:B],
                             start=False, stop=True)

            # --- partial logits -> DRAM -> ReduceScatter(add) ---
            acc_sb = sb1.tile([1, B], bf16)
            nc.vector.tensor_copy(out=acc_sb[:, 0:B // 2], in_=psA[0:1, :])
            nc.scalar.copy(out=acc_sb[:, B // 2:B], in_=psB[0:1, :])
            nc.sync.dma_start(accd[:], acc_sb[:])
            nc.gpsimd.collective_compute(
                "ReduceScatter",
                mybir.AluOpType.add,
                replica_groups=[list(range(N_CORES))],
                ins=[accd.opt()],
                outs=[rsd.opt()],
            )

            # --- sigmoid(logit/64 + b) for this core's 128 rows ---
            rs_sb = sb1.tile([1, B // N_CORES], bf16)
            nc.sync.dma_start(rs_sb[:], rsd[:])
            res = sb1.tile([1, B // N_CORES], f32)
            nc.scalar.activation(
                out=res[:], in_=rs_sb[:],
                func=mybir.ActivationFunctionType.Sigmoid,
                bias=b_sb[:], scale=1.0 / USCALE)
            nc.scalar.dma_start(outp.ap(), res[:])

    nc.compile()
    return nc


def kernel(x, emb_table, W, b):
    global _BUILT, LAST_RUN
    if _BUILT is None:
        _BUILT = _build()
    nc = _BUILT

    x = np.asarray(x)
    emb_table = np.ascontiguousarray(np.asarray(emb_table, dtype=np.float32))
    W = np.asarray(W, dtype=np.float32)
    b = np.asarray(b, dtype=np.float32)

    wemb = np.ascontiguousarray(W[:, :E])                  # [1, E]
    wv_full = W[0, E:]                                     # [V]
    bias_np = b.reshape(1, 1)

    # token -> (core, k, p, row) index decomposition
    rows_i = np.repeat(np.arange(B), S)
    v = x.reshape(-1).astype(np.int64)
    core = v // VPC
    vloc = v - core * VPC
    kk = vloc // 128
    pp = vloc - kk * 128

    # duplicate detection: count per (core, row, vloc)
    key = (core * B + rows_i) * VPC + vloc
    ukey, cnt = np.unique(key, return_counts=True)
    dup_sel = cnt >= 2
    d_key = ukey[dup_sel]
    d_extra = (cnt[dup_sel] - 1).astype(np.float32)
    d_core = d_key // (B * VPC)
    d_row = (d_key // VPC) % B
    d_vloc = d_key % VPC

    in_maps = []
    for c in range(N_CORES):
        tmp = np.zeros((VPAD, E), dtype=np.float32)
        tmp[:VPC] = emb_table[c * VPC:(c + 1) * VPC]
        # partition-major [p, k, e]: vocab row 128k+p at [p, k]
        tbl_np = np.ascontiguousarray(
            tmp.reshape(KC, 128, E).transpose(1, 0, 2))
        wvs = np.zeros(VPAD, dtype=np.float32)
        wvs[:VPC] = wv_full[c * VPC:(c + 1) * VPC]
        wvoc_np = np.ascontiguousarray(wvs.reshape(KC, 128).T)  # [128, KC]

        m = core == c
        pres_np = np.zeros((128, KC, B), dtype=ml_dtypes.float8_e4m3)
        pres_np[pp[m], kk[m], rows_i[m]] = 1.0

        dm = d_core == c
        nd = int(dm.sum())
        assert nd <= DUPN, f"core {c}: {nd} dup slots > {DUPN}"
        # t table flat position (p-major [128, KC]) -> 256B gather block
        dp = d_vloc[dm] % 128
        dk = d_vloc[dm] // 128
        flat = dp * KC + dk
        blk = (flat // GBLK).astype(np.int16)
        off = flat % GBLK

        bidx = np.zeros(DUPN, dtype=np.int16)
        bidx[:nd] = blk
        s_all = np.arange(DUPN)
        w16 = np.zeros((16, DUPN // 16), dtype=np.int16)
        w16[s_all % 16, s_all // 16] = bidx[s_all]
        didx_np = np.tile(w16, (8, 1))                      # [128, DUPN//16]

        dwv_np = np.zeros((128, 1, GBLK), dtype=ml_dtypes.bfloat16)
        dwv_np[np.arange(nd), 0, off] = (USCALE * d_extra[dm]).astype(
            ml_dtypes.bfloat16)
        dspread_np = np.zeros((128, B), dtype=ml_dtypes.bfloat16)
        dspread_np[np.arange(nd), d_row[dm]] = 1.0

        in_maps.append({
            "tbl": tbl_np,
            "wemb": wemb,
            "wvoc": wvoc_np,
            "pres": pres_np,
            "didx": didx_np,
            "dwv": dwv_np,
            "dspread": dspread_np,
            "bias": bias_np,
        })

    LAST_RUN = run_bass_kernel_spmd(nc, in_maps, core_ids=list(range(N_CORES)))
    out = np.concatenate(
        [LAST_RUN.results[c]["outp"].reshape(B // N_CORES)
         for c in range(N_CORES)]
    )
    return out.reshape(B, 1)


# revision 46
# speedup vs baseline: 1.1559x; 1.1559x over previous
"""Trainium2 Bass kernel for nn_LogisticRegression (embedding_lookup).

Reference computation (B=1024, S=200, V=50000, E=300):
    x1 = one-hot presence over vocab (duplicates set once)      [B, V]
    emb_mean = mean(emb_table[x], axis=1)                       [B, E]
    logits = concat([emb_mean, x1]) @ W.T + b                   [B, 1]
    out = sigmoid(logits)

Algebraic restructure (never materializes x1 / feats):
    t[v]     = emb_table[v] . W[0, :E] / S
    w[v]     = W[0, E + v]
    logit[i] = sum_v pres[i,v]*(t[v]+w[v]) + sum_dups (cnt-pres)*t[v] + b
where pres is the 0/1 presence matrix and the dup correction covers the
rare tokens repeated within a row (~0.4 per row).

Device plan (single NEFF, SPMD on 8 cores, vocab-sharded):
  Core c owns vocab rows [c*6250, (c+1)*6250) padded to 6400 = 50*128.
  phase 1: stream the 7.7MB table slice; per column k a fused DVE
           scalar_tensor_tensor computes t[:, k] = sum_e tbl*We/S with
           accum_out; u = 64*(t + wvoc) cast to fp8e4 (the 64x scale
           keeps u in e4m3 normal range; undone in the final sigmoid).
  phase 2: logits as a DENSE matmul against a host-built fp8 presence
           matrix (exact 0/1 values): DoubleRow fp8 matmuls contract 256
           vocab rows per instruction,
              psum[1, 1024] += u8[:, k:k+2].T @ presT[..][128, 2, 1024]
           accumulating across 25 pairs in 2 PSUM banks.  This replaces
           the per-token SWDGE dma_gather of the v1 kernel (which
           serialized ~8.4ns/index on the GpSimd Q7 path = 216us).
           Both streams rotate over the three DMA-capable issue queues
           (sync/scalar/gpsimd, ~95GB/s each; the aggregate ~280GB/s is
           the per-core HBM fair share with all 8 cores streaming), all
           tbl chunks ahead of pres chunks within each queue.
  dup fix: one 128-slot dma_gather pulls 256B blocks of the t table,
           a DVE one-hot extract forms val[s] = 64*d_s*t[p_s], and a
           [128,1].T @ [128,1024] matmul spreads the corrections onto
           the same PSUM accumulators.
  finish:  ReduceScatter(add) of the [1024] partial logits (4KB); core c
           receives rows [128c, 128c+128), applies sigmoid(x/64 + b),
           and writes its 128 outputs.  Host concatenation is a plain
           reorder of integer-indexed slices.

Empirical ground rules for this stack (established by direct HW tests):
  * indirect_dma_start ~10ns/descriptor serialized; dma_gather ~8.4ns/idx
    on the GpSimd software-DGE path -> avoid bulk gathers entirely;
  * tensor_tensor_reduce / tensor_scalar(accum_out) crash the compiled
    NEFF; scalar_tensor_tensor(accum_out) works and is used here;
  * only sync/scalar/gpsimd engines can issue DMAs; one queue sustains
    ~95-150GB/s, aggregate ~280GB/s (per-core HBM fair share);
  * the collective stream has a fixed ~88us init under this runtime --
    work finishing earlier than that just waits for the ReduceScatter.

Host side only shards tensors and precomputes integer index data (the
presence bitmap, dup slots, gather block ids) from the int token ids,
then concatenates the per-core outputs.
"""

import sys

if "/opt/trn_rl_repo" not in sys.path:
    sys.path.insert(0, "/opt/trn_rl_repo")

# This image's antenv package lacks the optional axon_hooks module, but
# concourse.bass_utils imports it unconditionally on the BASS_TRACE path.
# Provide a compatible stub so tracing degrades gracefully instead of
# crashing; a harness may install a real hook via set_axon_ntff_profile_hook.
try:
    import antenv.axon_hooks  # noqa: F401
except ImportError:
    import types as _types

    import antenv as _antenv

    _hooks_mod = _types.ModuleType("antenv.axon_hooks")
    _hooks_mod._hook = None

    def _set_hook(h, _m=_hooks_mod):
        _m._hook = h

    def _get_hook(_m=_hooks_mod):
        return _m._hook

    _hooks_mod.set_axon_ntff_profile_hook = _set_hook
    _hooks_mod.get_axon_ntff_profile_hook = _get_hook
    sys.modules["antenv.axon_hooks"] = _hooks_mod
    _antenv.axon_hooks = _hooks_mod

import ml_dtypes
import numpy as np

from concourse import bacc, bass, mybir, tile
from concourse.bass_utils import run_bass_kernel_spmd

# Problem shapes (hardcoded per contract).
N_CORES = 8
B = 1024
S = 200
V = 50000
E = 300

VPC = V // N_CORES          # vocab rows per core = 6250
KC = 50                     # u columns; padded vocab rows = 128*KC = 6400
VPAD = KC * 128
CHUNKS = [2, 4, 4, 6, 6, 6, 6, 6, 6, 4]   # k-columns per chunk group (even)
DUPN = 128                  # dup-correction slots per core
GBLK = 64                   # f32 per 256B gather block
USCALE = 64.0               # fp8 scale for u (undone in final sigmoid)

_BUILT = None
LAST_RUN = None  # BassKernelResults of the most recent launch (for harness)


def _build():
    f32 = mybir.dt.float32
    bf16 = mybir.dt.bfloat16
    fp8 = mybir.dt.float8e4
    i16 = mybir.dt.int16
    nc = bacc.Bacc("TRN2", target_bir_lowering=False, debug=False,
                   num_devices=N_CORES)

    # tbl/pres are host-pretransposed to partition-major [128, KC, ...] so
    # each streamed chunk is one contiguous run per partition (128 DMA
    # descriptors instead of 128*tch; multi-row APs cost the issuing
    # engine ~2-5us each)
    tbl = nc.dram_tensor("tbl", [128, KC, E], f32, kind="ExternalInput")
    wemb = nc.dram_tensor("wemb", [1, E], f32, kind="ExternalInput")
    wvoc = nc.dram_tensor("wvoc", [128, KC], f32, kind="ExternalInput")
    pres = nc.dram_tensor("pres", [128, KC, B], fp8, kind="ExternalInput")
    didx = nc.dram_tensor("didx", [128, DUPN // 16], i16, kind="ExternalInput")
    dwv = nc.dram_tensor("dwv", [128, 1, GBLK], bf16, kind="ExternalInput")
    dspread = nc.dram_tensor("dspread", [128, B], bf16, kind="ExternalInput")
    bias = nc.dram_tensor("bias", [1, 1], f32, kind="ExternalInput")
    outp = nc.dram_tensor("outp", [1, B // N_CORES], f32, kind="ExternalOutput")

    with tile.TileContext(nc) as tc:
        with tc.tile_pool(name="dram", bufs=1, space="DRAM") as dram, \
             tc.tile_pool(name="sbuf", bufs=1) as sb1, \
             tc.tile_pool(name="tb", bufs=len(CHUNKS)) as tb, \
             tc.tile_pool(name="pr", bufs=len(CHUNKS)) as pr, \
             tc.tile_pool(name="uf", bufs=len(CHUNKS)) as uf, \
             tc.tile_pool(name="scr", bufs=2) as scr, \
             tc.tile_pool(name="ps", bufs=1, space="PSUM") as ps:
            t_dram = dram.tile([VPAD // GBLK, GBLK], f32)
            # the CC stream has a fixed ~88us init in this runtime
            # (measured: a warm-up RS with input ready at 3us still
            # executed at 89us), so compute ending ~75us is already under
            # the collective floor; the RS is the structural tail
            accd = dram.tile([1, B], f32)
            rsd = dram.tile([1, B // N_CORES], f32)

            # --- small input loads (spread off the big-stream queues) ---
            wemb_sb = sb1.tile([128, E], f32)
            nc.scalar.dma_start(wemb_sb[:], wemb.ap().partition_broadcast(128))
            # fold the 1/S of the sequence mean into the embedding weights
            nc.vector.tensor_scalar_mul(wemb_sb[:], wemb_sb[:], 1.0 / S)
            wvoc_sb = sb1.tile([128, KC], f32)
            nc.scalar.dma_start(wvoc_sb[:], wvoc.ap())
            didx_sb = sb1.tile([128, DUPN // 16], i16)
            nc.sync.dma_start(didx_sb[:], didx.ap())
            dwv_sb = sb1.tile([128, 1, GBLK], bf16)
            nc.sync.dma_start(dwv_sb[:], dwv.ap())
            b_sb = sb1.tile([1, 1], f32)
            nc.scalar.dma_start(b_sb[:], bias.ap())
            t_raw = sb1.tile([128, KC], f32)
            psA = ps.tile([2, B // 2], f32)
            psB = ps.tile([2, B // 2], f32)

            # --- issue both big streams upfront, rotated over the three
            # DMA-capable queues (sync / scalar / gpsimd); within each
            # queue all tbl chunks sit ahead of pres chunks, since phase 1
            # gates everything downstream
            rot = [nc.sync, nc.scalar, nc.gpsimd]
            tchunks, pchunks = [], []
            k0 = 0
            for ch, tch in enumerate(CHUNKS):
                chunk = tb.tile([128, tch, E], f32, tag="tblchunk")
                rot[ch % 3].dma_start(chunk[:], tbl.ap()[:, k0:k0 + tch, :])
                tchunks.append(chunk)
                k0 += tch
            k0 = 0
            for ch, tch in enumerate(CHUNKS):
                pchunk = pr.tile([128, tch, B], fp8, tag="preschunk")
                rot[(ch + 1) % 3].dma_start(
                    pchunk[:], pres.ap()[:, k0:k0 + tch, :])
                pchunks.append(pchunk)
                k0 += tch
            # prime the Activation engine's sigmoid table now that its DMA
            # issues are queued (the 2.5us ACT_TABLE_LOAD otherwise blocks
            # them, or lands on the tail)
            warm = scr.tile([1, 1], f32, tag="warm")
            nc.scalar.activation(
                out=warm[:], in_=b_sb[:],
                func=mybir.ActivationFunctionType.Sigmoid)

            k0 = 0
            for ch, tch in enumerate(CHUNKS):
                chunk = tchunks[ch]
                pchunk = pchunks[ch]
                for t in range(tch):
                    k = k0 + t
                    po = scr.tile([128, E], f32, tag="po")
                    nc.vector.scalar_tensor_tensor(
                        out=po[:], in0=chunk[:, t, :], scalar=1.0,
                        in1=wemb_sb[:],
                        op0=mybir.AluOpType.mult, op1=mybir.AluOpType.mult,
                        accum_out=t_raw[:, k:k + 1])
                # u = 64*(t + wvoc) for this group, cast to fp8 for the PE.
                # Per-group u tile in DoubleRow weight layout [p, ktile r,
                # local pair c, m]: value u(2c+r) at m=0, zeros at the dummy
                # m=1 column (the dual-fp8 LDWEIGHTS path requires
                # n_elem[2]==2 and a 16B-aligned k-tile stride; the 16-slot
                # c axis gives r-stride 32B).  A single shared tile would
                # serialize this group's cast behind the previous group's
                # matmuls (write-after-read), stalling the DVE on the pres
                # stream.
                u_g = uf.tile([128, 2, 16, 2], fp8, tag="ug")
                nc.gpsimd.memset(u_g[:], 0.0)
                sl = slice(k0, k0 + tch)
                usl = scr.tile([128, tch], f32, tag="usl")
                nc.vector.tensor_tensor(
                    out=usl[:], in0=t_raw[:, sl], in1=wvoc_sb[:, sl],
                    op=mybir.AluOpType.add)
                for t in range(0, tch, 2):
                    nc.vector.tensor_scalar_mul(
                        u_g[:, :, t // 2, 0:1], usl[:, t:t + 2].unsqueeze(2),
                        USCALE)
                for t in range(0, tch, 2):
                    lhs = u_g[:, :, t // 2, :]
                    nc.tensor.matmul(
                        psA[:], lhs, pchunk[:, t:t + 2, 0:B // 2],
                        start=(k0 + t == 0), stop=False,
                        perf_mode=mybir.MatmulPerfMode.DoubleRow)
                    nc.tensor.matmul(
                        psB[:], lhs, pchunk[:, t:t + 2, B // 2:B],
                        start=(k0 + t == 0), stop=False,
                        perf_mode=mybir.MatmulPerfMode.DoubleRow)
                k0 += tch

            # --- dup correction: gather 256B t-blocks, extract, spread ---
            nc.sync.dma_start(
                t_dram[:].rearrange("q e -> (q e)").rearrange("(p k) -> p k", k=KC),
                t_raw[:])
            # dspread arrives late on the gpsimd queue; it is only needed by
            # the closing matmuls below
            dspread_sb = sb1.tile([128, B], bf16)
            nc.gpsimd.dma_start(dspread_sb[:], dspread.ap())
            g = sb1.tile([128, 1, GBLK], f32)
            nc.gpsimd.dma_gather(
                g[:], t_dram[:], didx_sb[:],
                num_idxs=DUPN, num_idxs_reg=DUPN, elem_size=GBLK)
            dpo = scr.tile([128, 1, GBLK], f32, tag="dpo")
            dval = sb1.tile([128, 1], f32)
            nc.vector.scalar_tensor_tensor(
                out=dpo[:], in0=g[:], scalar=1.0, in1=dwv_sb[:],
                op0=mybir.AluOpType.mult, op1=mybir.AluOpType.mult,
                accum_out=dval[:])
            dval_bf = sb1.tile([128, 1], bf16)
            nc.vector.tensor_copy(out=dval_bf[:], in_=dval[:])
            nc.tensor.matmul(psA[0:1, :], dval_bf[:], dspread_sb[:, 0:B // 2],
                             start=False, stop=True)
            nc.tensor.matmul(psB[0:1, :], dval_bf[:], dspread_sb[:, B // 2:B],
                             start=False, stop=True)

            # --- partial logits -> DRAM -> ReduceScatter(add) ---
            acc_sb = sb1.tile([1, B], f32)
            nc.vector.tensor_copy(out=acc_sb[:, 0:B // 2], in_=psA[0:1, :])
            nc.scalar.copy(out=acc_sb[:, B // 2:B], in_=psB[0:1, :])
            nc.sync.dma_start(accd[:], acc_sb[:])
            nc.gpsimd.collective_compute(
                "ReduceScatter",
                mybir.AluOpType.add,
                replica_groups=[list(range(N_CORES))],
                ins=[accd.opt()],
                outs=[rsd.opt()],
            )

            # --- sigmoid(logit/64 + b) for this core's 128 rows ---
            rs_sb = sb1.tile([1, B // N_CORES], f32)
            nc.sync.dma_start(rs_sb[:], rsd[:])
            res = sb1.tile([1, B // N_CORES], f32)
            nc.scalar.activation(
                out=res[:], in_=rs_sb[:],
                func=mybir.ActivationFunctionType.Sigmoid,
                bias=b_sb[:], scale=1.0 / USCALE)
            nc.scalar.dma_start(outp.ap(), res[:])

    nc.compile()
    return nc


def kernel(x, emb_table, W, b):
    global _BUILT, LAST_RUN
    if _BUILT is None:
        _BUILT = _build()
    nc = _BUILT

    x = np.asarray(x)
    emb_table = np.ascontiguousarray(np.asarray(emb_table, dtype=np.float32))
    W = np.asarray(W, dtype=np.float32)
    b = np.asarray(b, dtype=np.float32)

    wemb = np.ascontiguousarray(W[:, :E])                  # [1, E]
    wv_full = W[0, E:]                                     # [V]
    bias_np = b.reshape(1, 1)

    # token -> (core, k, p, row) index decomposition
    rows_i = np.repeat(np.arange(B), S)
    v = x.reshape(-1).astype(np.int64)
    core = v // VPC
    vloc = v - core * VPC
    kk = vloc // 128
    pp = vloc - kk * 128

    # duplicate detection: count per (core, row, vloc)
    key = (core * B + rows_i) * VPC + vloc
    ukey, cnt = np.unique(key, return_counts=True)
    dup_sel = cnt >= 2
    d_key = ukey[dup_sel]
    d_extra = (cnt[dup_sel] - 1).astype(np.float32)
    d_core = d_key // (B * VPC)
    d_row = (d_key // VPC) % B
    d_vloc = d_key % VPC

    in_maps = []
    for c in range(N_CORES):
        tmp = np.zeros((VPAD, E), dtype=np.float32)
        tmp[:VPC] = emb_table[c * VPC:(c + 1) * VPC]
        # partition-major [p, k, e]: vocab row 128k+p at [p, k]
        tbl_np = np.ascontiguousarray(
            tmp.reshape(KC, 128, E).transpose(1, 0, 2))
        wvs = np.zeros(VPAD, dtype=np.float32)
        wvs[:VPC] = wv_full[c * VPC:(c + 1) * VPC]
        wvoc_np = np.ascontiguousarray(wvs.reshape(KC, 128).T)  # [128, KC]

        m = core == c
        pres_np = np.zeros((128, KC, B), dtype=ml_dtypes.float8_e4m3)
        pres_np[pp[m], kk[m], rows_i[m]] = 1.0

        dm = d_core == c
        nd = int(dm.sum())
        assert nd <= DUPN, f"core {c}: {nd} dup slots > {DUPN}"
        # t table flat position (p-major [128, KC]) -> 256B gather block
        dp = d_vloc[dm] % 128
        dk = d_vloc[dm] // 128
        flat = dp * KC + dk
        blk = (flat // GBLK).astype(np.int16)
        off = flat % GBLK

        bidx = np.zeros(DUPN, dtype=np.int16)
        bidx[:nd] = blk
        s_all = np.arange(DUPN)
        w16 = np.zeros((16, DUPN // 16), dtype=np.int16)
        w16[s_all % 16, s_all // 16] = bidx[s_all]
        didx_np = np.tile(w16, (8, 1))                      # [128, DUPN//16]

        dwv_np = np.zeros((128, 1, GBLK), dtype=ml_dtypes.bfloat16)
        dwv_np[np.arange(nd), 0, off] = (USCALE * d_extra[dm]).astype(
            ml_dtypes.bfloat16)
        dspread_np = np.zeros((128, B), dtype=ml_dtypes.bfloat16)
        dspread_np[np.arange(nd), d_row[dm]] = 1.0

        in_maps.append({
            "tbl": tbl_np,
            "wemb": wemb,
            "wvoc": wvoc_np,
            "pres": pres_np,
            "didx": didx_np,
            "dwv": dwv_np,
            "dspread": dspread_np,
            "bias": bias_np,
        })

    LAST_RUN = run_bass_kernel_spmd(nc, in_maps, core_ids=list(range(N_CORES)))
    out = np.concatenate(
        [LAST_RUN.results[c]["outp"].reshape(B // N_CORES)
         for c in range(N_CORES)]
    )
    return out.reshape(B, 1)
